# revision 12
# baseline (speedup 1.0000x reference)
"""Distributed Trainium2 kernel for a decoder prompt layer (8 NeuronCores).

Sharding: heads (tensor-parallel) for QKV/attention, tokens for the
out-projection + FFN tail, joined by one AllToAll of the attention output.
"""

import sys

sys.path.insert(0, "/opt/trn_rl_repo")

import numpy as np
import ml_dtypes

import concourse.bass as bass
import concourse.mybir as mybir
import concourse.tile as tile
from concourse import bacc
from concourse import bass_utils

BF16 = ml_dtypes.bfloat16

B, T, M, D, H, DH, DF = 2, 2048, 256, 1024, 16, 64, 4096
R = DH // 2  # 32, rotary dims per head
S = M + T  # 2304
W = 8  # cores
EPS = 1e-5
BT = B * T  # 4096 flat decoder tokens
BS = B * S  # 4608 flat full tokens
TPC = BT // W  # 512 tail tokens per core
HPC = H // W  # 2 heads per core
NKB = S // 128  # 18 key blocks per batch
NQB = T // 512  # 4 query blocks per batch
DB = D // 128  # 8 contraction blocks of D
FB = DF // 128  # 32 blocks of DF

F32 = mybir.dt.float32
BF = mybir.dt.bfloat16
ACTF = mybir.ActivationFunctionType

KERNEL_STATS = {}

_CACHED_NC = None
DEBUG = False


def _layernorm_tiles(nc, stat_pool, eps_t, y_t, xf_t, rstd_out=None):
    """Emit LN over a (128, 1024) f32 tile y_t -> normalized (no g/b) xf_t."""
    st = stat_pool.tile([128, 2, 6], F32, tag="bnst")
    nc.vector.bn_stats(out=st[:, 0, :], in_=y_t[:, 0:512])
    nc.vector.bn_stats(out=st[:, 1, :], in_=y_t[:, 512:1024])
    mv = stat_pool.tile([128, 2], F32, tag="mv")
    nc.vector.bn_aggr(out=mv[:], in_=st[:])
    rstd = stat_pool.tile([128, 1], F32, tag="rstd")
    nc.scalar.activation(
        out=rstd[:], in_=mv[:, 1:2], func=ACTF.Sqrt, bias=eps_t[:], scale=1.0
    )
    nc.vector.reciprocal(out=rstd[:], in_=rstd[:])
    nmr = stat_pool.tile([128, 1], F32, tag="nmr")
    nc.vector.tensor_mul(out=nmr[:], in0=mv[:, 0:1], in1=rstd[:])
    nc.vector.tensor_scalar_mul(out=nmr[:], in0=nmr[:], scalar1=-1.0)
    nc.scalar.activation(
        out=xf_t[:], in_=y_t[:], func=ACTF.Identity, bias=nmr[:], scale=rstd[:]
    )


def _build_nc():
    nc = bacc.Bacc(trn_type="TRN2", debug=False, num_devices=W)

    io = {}
    io["xfull"] = nc.dram_tensor("xfull", [BS, D], F32, kind="ExternalInput")
    for n in ("wq", "wk", "wv"):
        io[n] = nc.dram_tensor(n, [128, DB * 128], BF, kind="ExternalInput")
    for n in ("bq", "bk", "bv"):
        io[n] = nc.dram_tensor(n, [128, 1], F32, kind="ExternalInput")
    io["cos_k"] = nc.dram_tensor("cos_k", [128, BS], BF, kind="ExternalInput")
    io["sin_k"] = nc.dram_tensor("sin_k", [128, BS], BF, kind="ExternalInput")
    io["cos_q"] = nc.dram_tensor("cos_q", [128, BT], BF, kind="ExternalInput")
    io["sin_q"] = nc.dram_tensor("sin_q", [128, BT], BF, kind="ExternalInput")
    io["masks"] = nc.dram_tensor("masks", [4, 128, 512], BF, kind="ExternalInput")
    io["wo"] = nc.dram_tensor("wo", [128, DB * D], BF, kind="ExternalInput")
    io["w1"] = nc.dram_tensor("w1", [FB, 128, DB * 128], BF, kind="ExternalInput")
    io["b1"] = nc.dram_tensor("b1", [128, FB], F32, kind="ExternalInput")
    io["w2"] = nc.dram_tensor("w2", [FB, 128, D], BF, kind="ExternalInput")
    io["b2"] = nc.dram_tensor("b2", [1, D], F32, kind="ExternalInput")
    io["xres"] = nc.dram_tensor("xres", [TPC, D], F32, kind="ExternalInput")
    io["out"] = nc.dram_tensor("out", [TPC, D], F32, kind="ExternalOutput")

    if DEBUG:
        io["dbg_xa"] = nc.dram_tensor("dbg_xa", [BS, D], BF, kind="ExternalOutput")
        io["dbg_q"] = nc.dram_tensor("dbg_q", [128, BT], BF, kind="ExternalOutput")
        io["dbg_k"] = nc.dram_tensor("dbg_k", [128, BS], BF, kind="ExternalOutput")
        io["dbg_v"] = nc.dram_tensor("dbg_v", [128, BS], BF, kind="ExternalOutput")
        io["dbg_ai"] = nc.dram_tensor("dbg_ai", [W * 128, TPC], BF, kind="ExternalOutput")
        io["dbg_ao"] = nc.dram_tensor("dbg_ao", [W * 128, TPC], BF, kind="ExternalOutput")
        io["dbg_y"] = nc.dram_tensor("dbg_y", [TPC, D], F32, kind="ExternalOutput")
        io["dbg_at"] = nc.dram_tensor("dbg_at", [6, 128, 512], BF, kind="ExternalOutput")
        io["dbg_po"] = nc.dram_tensor("dbg_po", [65, 512], F32, kind="ExternalOutput")
        io["dbg_vtok"] = nc.dram_tensor("dbg_vtok", [128, NKB * 160], BF, kind="ExternalOutput")
    io["xa_dram"] = nc.dram_tensor("xa_scratch", [BS, D], BF)
    io["xf_dram"] = nc.dram_tensor("xf_scratch", [TPC, D], BF)
    io["a2a_in"] = nc.dram_tensor("a2a_in", [W * 128, TPC], BF)
    io["a2a_out"] = nc.dram_tensor("a2a_out", [W * 128, TPC], BF)

    with tile.TileContext(nc) as tc:
        _emit(nc, tc, io)
    nc.compile()
    return nc


def _emit(nc, tc, io):
    xfull = io["xfull"].ap()
    xa_dram = io["xa_dram"].ap()
    xf_dram = io["xf_dram"].ap()
    out = io["out"].ap()
    xres = io["xres"].ap()

    # ---- persistent pools (whole kernel) ----
    late_cm = tc.tile_pool(name="late", bufs=1)
    stat_cm = tc.tile_pool(name="stats", bufs=8)
    psum_cm = tc.tile_pool(name="psum", bufs=8, space="PSUM")
    late = late_cm.__enter__()
    stat_pool = stat_cm.__enter__()
    psum = psum_cm.__enter__()

    eps_t = late.tile([128, 1], F32, tag="eps")
    nc.vector.memset(eps_t, EPS)
    b1_t = late.tile([128, FB], F32, tag="b1")
    nc.sync.dma_start(out=b1_t[:], in_=io["b1"].ap())
    b2row = late.tile([1, D], F32, tag="b2row")
    nc.sync.dma_start(out=b2row[:], in_=io["b2"].ap())
    b2_t = late.tile([128, D], F32, tag="b2b")
    nc.gpsimd.partition_broadcast(out_ap=b2_t[:], in_ap=b2row[:])
    wo_t = late.tile([128, DB, D], BF, tag="wo")
    nc.sync.dma_start(
        out=wo_t[:], in_=io["wo"].ap().rearrange("p (a c) -> p a c", c=D)
    )

    # ---- early constants (attention phase) ----
    early_cm = tc.tile_pool(name="early", bufs=1)
    early = early_cm.__enter__()
    b_tiles = {}
    for n in ("bq", "bk", "bv"):
        b_tiles[n] = early.tile([128, 1], F32, tag=n, name=n + "_t")
        nc.sync.dma_start(out=b_tiles[n][:], in_=io[n].ap())
    cosk_t = early.tile([128, BS], BF, tag="cosk")
    sink_t = early.tile([128, BS], BF, tag="sink")
    cosq_t = early.tile([128, BT], BF, tag="cosq")
    sinq_t = early.tile([128, BT], BF, tag="sinq")
    nc.sync.dma_start(out=cosk_t[:], in_=io["cos_k"].ap())
    nc.sync.dma_start(out=sink_t[:], in_=io["sin_k"].ap())
    nc.sync.dma_start(out=cosq_t[:], in_=io["cos_q"].ap())
    nc.sync.dma_start(out=sinq_t[:], in_=io["sin_q"].ap())
    mask_t = early.tile([128, 4, 512], BF, tag="masks")
    nc.sync.dma_start(out=mask_t[:], in_=io["masks"].ap().rearrange("j p q -> p j q"))
    w_tiles = {}
    for n in ("wq", "wk", "wv"):
        w_tiles[n] = early.tile([128, DB, 128], BF, tag=n, name=n + "_t")
        nc.sync.dma_start(
            out=w_tiles[n][:],
            in_=io[n].ap().rearrange("p (a c) -> p a c", c=128),
        )

    # ---------- phase A: LN1 over all BS tokens ----------
    ln_cm = tc.tile_pool(name="ln", bufs=4)
    ln_pool = ln_cm.__enter__()
    for ch in range(BS // 128):
        x_t = ln_pool.tile([128, D], F32, tag="xin")
        nc.sync.dma_start(out=x_t[:], in_=xfull[ch * 128 : (ch + 1) * 128, :])
        xa_t = ln_pool.tile([128, D], BF, tag="xaout")
        _layernorm_tiles(nc, stat_pool, eps_t, x_t, xa_t)
        nc.sync.dma_start(out=xa_dram[ch * 128 : (ch + 1) * 128, :], in_=xa_t[:])
    ln_cm.__exit__(None, None, None)

    # ---------- phase B: transpose xa -> xaT (d-major) ----------
    xaT_cm = tc.tile_pool(name="xaT", bufs=1, side="right")
    xaT_pool = xaT_cm.__enter__()
    xaT = xaT_pool.tile([128, DB, BS], BF, tag="xaT")
    for db in range(DB):
        nc.sync.dma_start(
            out=xaT[:, db, :],
            in_=xa_dram[:, db * 128 : (db + 1) * 128],
            transpose=True,
        )

    # ---------- phase C: QKV projections (d-major outputs) ----------
    qkv_cm = tc.tile_pool(name="qkvT", bufs=1)
    qkv_pool = qkv_cm.__enter__()
    qT = qkv_pool.tile([128, BT], BF, tag="qT")
    kT = qkv_pool.tile([128, BS], BF, tag="kT")
    vT = qkv_pool.tile([128, BS], BF, tag="vT")

    def proj(w_tile, bias_t, dst, ranges):
        for grp in range(0, len(ranges), 4):
            chunk = ranges[grp : grp + 4]
            ps = [
                psum.tile([128, c[0][1] - c[0][0]], F32, tag="ps", name="ps_proj") for c in chunk
            ]
            for db in range(DB):
                for i, (src, _d) in enumerate(chunk):
                    nc.tensor.matmul(
                        ps[i][:],
                        w_tile[:, db, :],
                        xaT[:, db, src[0] : src[1]],
                        start=(db == 0),
                        stop=(db == DB - 1),
                    )
            for i, (_s, dstc) in enumerate(chunk):
                nc.vector.tensor_scalar_add(
                    out=dst[:, dstc[0] : dstc[1]], in0=ps[i][:], scalar1=bias_t[:]
                )

    k_ranges = [
        ((i * 512, (i + 1) * 512), (i * 512, (i + 1) * 512))
        for i in range(BS // 512)
    ]
    q_ranges = []
    for b in range(B):
        for i in range(T // 512):
            src0 = b * S + M + i * 512
            dst0 = b * T + i * 512
            q_ranges.append(((src0, src0 + 512), (dst0, dst0 + 512)))
    proj(w_tiles["wk"], b_tiles["bk"], kT, k_ranges)
    proj(w_tiles["wv"], b_tiles["bv"], vT, k_ranges)
    proj(w_tiles["wq"], b_tiles["bq"], qT, q_ranges)

    # ---------- RoPE on q/k (first R dims of each head), 512-col chunks ----
    rope_cm = tc.tile_pool(name="rope", bufs=4, side="right")
    rope_pool = rope_cm.__enter__()

    def rope(dst, cos_t, sin_t, n):
        for h in range(HPC):
            r0 = h * DH
            for c0 in range(0, n, 512):
                c1 = c0 + 512
                rot = rope_pool.tile([128, 512], BF, tag="rot")
                t1 = rope_pool.tile([128, 512], BF, tag="t1")
                rr = slice(r0, r0 + R)
                nc.sync.dma_start(
                    out=rot[r0 : r0 + 16, :], in_=dst[r0 + 16 : r0 + 32, c0:c1]
                )
                nc.sync.dma_start(
                    out=rot[r0 + 16 : r0 + 32, :], in_=dst[r0 : r0 + 16, c0:c1]
                )
                nc.vector.tensor_mul(
                    out=t1[rr, :], in0=dst[rr, c0:c1], in1=cos_t[rr, c0:c1]
                )
                nc.vector.tensor_mul(
                    out=rot[rr, :], in0=rot[rr, :], in1=sin_t[rr, c0:c1]
                )
                nc.vector.tensor_add(
                    out=dst[rr, c0:c1], in0=t1[rr, :], in1=rot[rr, :]
                )

    rope(kT, cosk_t, sink_t, BS)
    rope(qT, cosq_t, sinq_t, BT)
    rope_cm.__exit__(None, None, None)
    xaT_cm.__exit__(None, None, None)

    if DEBUG:
        nc.sync.dma_start(out=io["dbg_xa"].ap(), in_=xa_dram[:, :])
        nc.sync.dma_start(out=io["dbg_q"].ap(), in_=qT[:])
        nc.sync.dma_start(out=io["dbg_k"].ap(), in_=kT[:])
        nc.sync.dma_start(out=io["dbg_v"].ap(), in_=vT[:])

    # ---------- phase D: V -> token-major with ones columns ----------
    vtok_cm = tc.tile_pool(name="vtok", bufs=1)
    vtok_pool = vtok_cm.__enter__()
    vtok = [vtok_pool.tile([128, NKB, 2, 80], BF, tag=f"vtok{b}", name=f"vtok{b}") for b in range(B)]
    for b in range(B):
        nc.vector.memset(vtok[b][:, :, :, 64:65], 1.0)
        for kb in range(NKB):
            c0 = b * S + kb * 128
            for h in range(HPC):
                nc.sync.dma_start(
                    out=vtok[b][:, kb, h, 0:64],
                    in_=vT[h * DH : (h + 1) * DH, c0 : c0 + 128],
                    transpose=True,
                )

    if DEBUG:
        nc.sync.dma_start(
            out=io["dbg_vtok"].ap(),
            in_=vtok[0][:].rearrange("p a h c -> p (a h c)"),
        )

    # ---------- phase E: attention ----------
    attn_cm = tc.tile_pool(name="attnT", bufs=6)
    attn_pool = attn_cm.__enter__()
    nrm_cm = tc.tile_pool(name="nrm", bufs=4)
    nrm_pool = nrm_cm.__enter__()  # pop order: nrm, attn, vtok, qkv
    a2a_in = io["a2a_in"]
    a2a_out = io["a2a_out"]
    a2a_in_v = a2a_in.ap().rearrange("(j p) q -> j p q", p=128)
    for b in range(B):
        for qb in range(NQB):
            nk = M // 128 + 4 * (qb + 1)
            qc0 = b * T + qb * 512
            po = [psum.tile([65, 512], F32, tag="ps", name="po") for _ in range(HPC)]
            for kb in range(nk):
                kc0 = b * S + kb * 128
                pss = [psum.tile([128, 512], F32, tag="ps", name="pss") for _ in range(HPC)]
                for h in range(HPC):
                    h0 = h * DH
                    nc.tensor.matmul(
                        pss[h][:],
                        kT[h0 : h0 + DH, kc0 : kc0 + 128],
                        qT[h0 : h0 + DH, qc0 : qc0 + 512],
                        start=True,
                        stop=True,
                    )
                at = []
                for h in range(HPC):
                    a = attn_pool.tile([128, 512], BF, tag="at")
                    nc.scalar.activation(out=a[:], in_=pss[h][:], func=ACTF.Exp)
                    j = kb - (nk - 4)
                    if j >= 0:
                        nc.vector.tensor_mul(out=a[:], in0=a[:], in1=mask_t[:, j, :])
                    at.append(a)
                    if DEBUG and b == 0 and qb == 0 and h == 0:
                        nc.sync.dma_start(
                            out=io["dbg_at"].ap()[kb], in_=a[:]
                        )
                for h in range(HPC):
                    nc.tensor.matmul(
                        po[h][:],
                        vtok[b][:, kb, h, 0:65],
                        at[h][:],
                        start=(kb == 0),
                        stop=(kb == nk - 1),
                    )
            j = b * NQB + qb
            if DEBUG and b == 0 and qb == 0:
                podbg = nrm_pool.tile([65, 512], F32, tag="podbg")
                nc.vector.tensor_copy(out=podbg[:], in_=po[0][:])
                nc.sync.dma_start(out=io["dbg_po"].ap(), in_=podbg[:])
            for h in range(HPC):
                rec64 = nrm_pool.tile([65, 512], F32, tag="rec64")
                nc.vector.reciprocal(out=rec64[64:65, :], in_=po[h][64:65, :])
                rec0 = nrm_pool.tile([1, 512], F32, tag="rec0")
                nc.sync.dma_start(out=rec0[:], in_=rec64[64:65, :])
                recb = nrm_pool.tile([64, 512], F32, tag="recb")
                nc.gpsimd.partition_broadcast(out_ap=recb[:], in_ap=rec0[:])
                onorm = nrm_pool.tile([64, 512], BF, tag="onorm")
                nc.vector.tensor_mul(
                    out=onorm[:, :], in0=po[h][0:64, :], in1=recb[:, :]
                )
                nc.sync.dma_start(
                    out=a2a_in_v[j, h * DH : (h + 1) * DH, :], in_=onorm[:, :]
                )
    # ---------- phase F: all-to-all ----------
    nc.gpsimd.collective_compute(
        "AllToAll",
        mybir.AluOpType.bypass,
        replica_groups=[list(range(W))],
        ins=[a2a_in.ap()],
        outs=[a2a_out.ap()],
    )
    nrm_cm.__exit__(None, None, None)
    attn_cm.__exit__(None, None, None)
    vtok_cm.__exit__(None, None, None)
    qkv_cm.__exit__(None, None, None)
    early_cm.__exit__(None, None, None)
    ofT_cm = tc.tile_pool(name="ofT", bufs=1)
    ofT_pool = ofT_cm.__enter__()
    ofT = ofT_pool.tile([128, W, TPC], BF, tag="ofT")
    nc.sync.dma_start(
        out=ofT[:], in_=a2a_out.ap().rearrange("(j p) q -> p j q", p=128)
    )

    if DEBUG:
        nc.sync.dma_start(out=io["dbg_ai"].ap(), in_=a2a_in.ap())
        nc.sync.dma_start(out=io["dbg_ao"].ap(), in_=a2a_out.ap())

    # ---------- phase G: out-proj + residual + LN2 ----------
    y_cm = tc.tile_pool(name="y", bufs=1)
    y_pool = y_cm.__enter__()
    xr_cm = tc.tile_pool(name="xr", bufs=2)
    xr_pool = xr_cm.__enter__()
    ln2_cm = tc.tile_pool(name="ln2", bufs=4)
    ln2_pool = ln2_cm.__enter__()
    y_tiles = []
    for tt in range(TPC // 128):
        xr_t = xr_pool.tile([128, D], F32, tag="xr")
        nc.sync.dma_start(out=xr_t[:], in_=xres[tt * 128 : (tt + 1) * 128, :])
        pz = [psum.tile([128, 512], F32, tag="ps", name="pz") for _ in range(2)]
        for db in range(DB):
            for half in range(2):
                nc.tensor.matmul(
                    pz[half][:],
                    ofT[:, db, tt * 128 : (tt + 1) * 128],
                    wo_t[:, db, half * 512 : (half + 1) * 512],
                    start=(db == 0),
                    stop=(db == DB - 1),
                )
        y_t = y_pool.tile([128, D], F32, tag=f"y{tt}", name=f"y{tt}")
        for half in range(2):
            hs = slice(half * 512, (half + 1) * 512)
            nc.vector.tensor_add(out=y_t[:, hs], in0=pz[half][:], in1=xr_t[:, hs])
        y_tiles.append(y_t)
        if DEBUG:
            nc.sync.dma_start(
                out=io["dbg_y"].ap()[tt * 128 : (tt + 1) * 128, :], in_=y_t[:]
            )
        xf_t = ln2_pool.tile([128, D], BF, tag="xf")
        _layernorm_tiles(nc, stat_pool, eps_t, y_t, xf_t)
        nc.sync.dma_start(out=xf_dram[tt * 128 : (tt + 1) * 128, :], in_=xf_t[:])
    ln2_cm.__exit__(None, None, None)
    xr_cm.__exit__(None, None, None)

    # ---------- phase H: transpose xf ----------
    xfT_cm = tc.tile_pool(name="xfT", bufs=1)
    xfT_pool = xfT_cm.__enter__()
    xfT = xfT_pool.tile([128, DB, TPC], BF, tag="xfT")
    for db in range(DB):
        nc.sync.dma_start(
            out=xfT[:, db, :],
            in_=xf_dram[:, db * 128 : (db + 1) * 128],
            transpose=True,
        )

    # ---------- phase I: FFN1 (h = relu(xf@W1+b1)^2, DF-major) ----------
    h2_cm = tc.tile_pool(name="h2T", bufs=1)
    h2_pool = h2_cm.__enter__()
    w2_cm = tc.tile_pool(name="w2full", bufs=1)
    w2_pool = w2_cm.__enter__()
    w1_cm = tc.tile_pool(name="w1c", bufs=4)
    w1_pool = w1_cm.__enter__()
    hr_cm = tc.tile_pool(name="hr", bufs=4)
    hr_pool = hr_cm.__enter__()
    w2full = w2_pool.tile([128, FB, D], BF, tag="w2full")
    nc.sync.dma_start(out=w2full[:], in_=io["w2"].ap().rearrange("f p c -> p f c"))
    h2T = h2_pool.tile([128, FB, TPC], BF, tag="h2T")
    w1_ap = io["w1"].ap()
    for fb in range(FB):
        w1c = w1_pool.tile([128, DB, 128], BF, tag="w1c")
        nc.sync.dma_start(
            out=w1c[:], in_=w1_ap[fb].rearrange("p (a c) -> p a c", c=128)
        )
        ph = psum.tile([128, TPC], F32, tag="ps")
        for db in range(DB):
            nc.tensor.matmul(
                ph[:],
                w1c[:, db, :],
                xfT[:, db, :],
                start=(db == 0),
                stop=(db == DB - 1),
            )
        hr = hr_pool.tile([128, TPC], BF, tag="hr")
        nc.scalar.activation(
            out=hr[:],
            in_=ph[:],
            func=ACTF.Relu,
            bias=b1_t[:, fb : fb + 1],
            scale=1.0,
        )
        nc.vector.tensor_mul(out=h2T[:, fb, :], in0=hr[:], in1=hr[:])
    hr_cm.__exit__(None, None, None)
    w1_cm.__exit__(None, None, None)

    # ---------- phase J: FFN2 + residual + store ----------
    out_cm = tc.tile_pool(name="outp", bufs=2)
    out_pool = out_cm.__enter__()
    for tt in range(TPC // 128):
        pz = [psum.tile([128, 512], F32, tag="ps", name="pz") for _ in range(2)]
        for fb in range(FB):
            for half in range(2):
                nc.tensor.matmul(
                    pz[half][:],
                    h2T[:, fb, tt * 128 : (tt + 1) * 128],
                    w2full[:, fb, half * 512 : (half + 1) * 512],
                    start=(fb == 0),
                    stop=(fb == FB - 1),
                )
        o_t = out_pool.tile([128, D], F32, tag="ot")
        for half in range(2):
            hs = slice(half * 512, (half + 1) * 512)
            nc.vector.tensor_add(
                out=o_t[:, hs], in0=pz[half][:], in1=y_tiles[tt][:, hs]
            )
            nc.vector.tensor_add(out=o_t[:, hs], in0=o_t[:, hs], in1=b2_t[:, hs])
        nc.sync.dma_start(out=out[tt * 128 : (tt + 1) * 128, :], in_=o_t[:])
    out_cm.__exit__(None, None, None)
    w2_cm.__exit__(None, None, None)
    h2_cm.__exit__(None, None, None)
    xfT_cm.__exit__(None, None, None)
    y_cm.__exit__(None, None, None)
    ofT_cm.__exit__(None, None, None)
    stat_cm.__exit__(None, None, None)
    late_cm.__exit__(None, None, None)
    psum_cm.__exit__(None, None, None)


def _prep_inputs(x, memory, Wq, bq, Wk, bk, Wv, bv, Wo, bo, W1, b1, W2, b2,
                 ln1_g, ln1_b, ln2_g, ln2_b):
    f32 = np.float32
    x = np.asarray(x, f32)
    memory = np.asarray(memory, f32)
    x_full = np.concatenate([memory, x], axis=1).reshape(BS, D)

    g1 = np.asarray(ln1_g, f32)
    b1n = np.asarray(ln1_b, f32)
    g2 = np.asarray(ln2_g, f32)
    b2n = np.asarray(ln2_b, f32)

    scale_q = np.float32(1.0 / np.sqrt(np.float32(DH)))
    Wq_e = (g1[:, None] * np.asarray(Wq, f32)) * scale_q
    bq_e = (b1n @ np.asarray(Wq, f32) + np.asarray(bq, f32)) * scale_q
    Wk_e = g1[:, None] * np.asarray(Wk, f32)
    bk_e = b1n @ np.asarray(Wk, f32) + np.asarray(bk, f32)
    Wv_e = g1[:, None] * np.asarray(Wv, f32)
    bv_e = b1n @ np.asarray(Wv, f32) + np.asarray(bv, f32)
    W1_e = g2[:, None] * np.asarray(W1, f32)
    b1_e = b2n @ np.asarray(W1, f32) + np.asarray(b1, f32)

    # rope tables (feature-major)
    inv_freq = 1.0 / (10000.0 ** (np.arange(0, R, 2, dtype=f32) / np.float32(R)))
    t = np.arange(S, dtype=f32)
    freqs = t[:, None] * inv_freq[None, :]
    emb = np.concatenate([freqs, freqs], axis=-1)  # (S, R)
    cos = np.cos(emb).T.astype(f32)  # (R, S)
    sin = np.sin(emb).T.astype(f32)
    ssin = np.concatenate([-sin[:16], sin[16:]], axis=0)

    def wide(tab):  # (R, n) -> (128, n) with copies at head offsets 0 and 64
        o = np.zeros((128, tab.shape[1]), np.float32)
        o[0:R] = tab
        o[64 : 64 + R] = tab
        return o

    cos_k = wide(np.tile(cos, (1, B))).astype(BF16)
    sin_k = wide(np.tile(ssin, (1, B))).astype(BF16)
    cos_q = wide(np.tile(cos[:, M:], (1, B))).astype(BF16)
    sin_q = wide(np.tile(ssin[:, M:], (1, B))).astype(BF16)

    # diagonal causal masks (key-block row j): allow q >= 128*j + k
    kk = np.arange(128)[:, None]
    qq = np.arange(512)[None, :]
    mask = np.stack([(qq >= 128 * j + kk) for j in range(4)]).astype(BF16)

    wo_host = (
        np.ascontiguousarray(
            np.asarray(Wo, f32).reshape(DB, 128, D).transpose(1, 0, 2)
        )
        .reshape(128, DB * D)
        .astype(BF16)
    )
    w1_host = (
        np.ascontiguousarray(W1_e.reshape(DB, 128, FB, 128).transpose(2, 1, 0, 3))
        .reshape(FB, 128, DB * 128)
        .astype(BF16)
    )
    w2_host = np.asarray(W2, f32).reshape(FB, 128, D).astype(BF16)
    b1_host = np.ascontiguousarray(b1_e.reshape(FB, 128).T).astype(f32)  # (128, FB)
    b2_host = np.ascontiguousarray(np.asarray(b2, f32).reshape(1, D))

    bo_arr = np.asarray(bo, f32)
    x_dec = x.reshape(BT, D)

    in_maps = []
    for c in range(W):
        cols = slice(c * 128, (c + 1) * 128)

        def wslice(We):
            return (
                np.ascontiguousarray(
                    We[:, cols].reshape(DB, 128, 128).transpose(1, 0, 2)
                )
                .reshape(128, DB * 128)
                .astype(BF16)
            )

        xres_c = x_dec[c * TPC : (c + 1) * TPC] + bo_arr[None, :]
        in_maps.append(
            {
                "xfull": x_full,
                "wq": wslice(Wq_e),
                "wk": wslice(Wk_e),
                "wv": wslice(Wv_e),
                "bq": np.ascontiguousarray(bq_e[cols].reshape(128, 1)),
                "bk": np.ascontiguousarray(bk_e[cols].reshape(128, 1)),
                "bv": np.ascontiguousarray(bv_e[cols].reshape(128, 1)),
                "cos_k": cos_k,
                "sin_k": sin_k,
                "cos_q": cos_q,
                "sin_q": sin_q,
                "masks": mask,
                "wo": wo_host,
                "w1": w1_host,
                "b1": b1_host,
                "w2": w2_host,
                "b2": b2_host,
                "xres": np.ascontiguousarray(xres_c, dtype=f32),
            }
        )
    return in_maps


def kernel(**inputs):
    global _CACHED_NC
    if _CACHED_NC is None:
        _CACHED_NC = _build_nc()
    nc = _CACHED_NC
    in_maps = _prep_inputs(**inputs)
    res = bass_utils.run_bass_kernel_spmd(nc, in_maps, core_ids=list(range(W)))
    KERNEL_STATS["exec_time_ns"] = res.exec_time_ns
    outs = np.stack([res.results[c]["out"] for c in range(W)])  # (W, TPC, D)
    return outs.reshape(B, T, D).astype(np.float32)


# revision 26
# speedup vs baseline: 1.4579x; 1.4579x over previous
"""Distributed Trainium2 kernel for a decoder prompt layer (8 NeuronCores).

Sharding: hybrid batch x head tensor-parallel attention (each core: 1 batch,
4 heads), token-parallel out-proj + FFN tail, joined by two concurrent 4-core
AllToAll collectives (one per batch group).
"""

import sys

sys.path.insert(0, "/opt/trn_rl_repo")

import numpy as np
import ml_dtypes

import concourse.bass as bass
import concourse.mybir as mybir
import concourse.tile as tile
from concourse import bacc
from concourse.masks import make_identity
from concourse import bass_utils

BF16 = ml_dtypes.bfloat16

B, T, M, D, H, DH, DF = 2, 2048, 256, 1024, 16, 64, 4096
R = DH // 2  # 32, rotary dims per head
S = M + T  # 2304
W = 8  # cores
EPS = 1e-5
BT = B * T  # 4096 flat decoder tokens
TPC = BT // W  # 512 tail tokens per core
NHC = 4  # heads per core (one batch per core)
NP = 2  # head pairs per core
NKB = S // 128  # 18 key blocks
NQB = T // 512  # 4 query blocks
DB = D // 128  # 8 contraction blocks of D
FB = DF // 128  # 32 blocks of DF

F32 = mybir.dt.float32
BF = mybir.dt.bfloat16
ACTF = mybir.ActivationFunctionType

KERNEL_STATS = {}

_CACHED_NC = None
DEBUG = False


def _layernorm_tiles(nc, stat_pool, eps_t, y_t, xf_t):
    """LN over a (128, 1024) tile y_t -> normalized (no g/b) xf_t."""
    st = stat_pool.tile([128, 2, 6], F32, tag="bnst")
    nc.vector.bn_stats(out=st[:, 0, :], in_=y_t[:, 0:512])
    nc.vector.bn_stats(out=st[:, 1, :], in_=y_t[:, 512:1024])
    mv = stat_pool.tile([128, 2], F32, tag="mv")
    nc.vector.bn_aggr(out=mv[:], in_=st[:])
    rstd = stat_pool.tile([128, 1], F32, tag="rstd")
    nc.scalar.activation(
        out=rstd[:], in_=mv[:, 1:2], func=ACTF.Sqrt, bias=eps_t[:], scale=1.0
    )
    nc.vector.reciprocal(out=rstd[:], in_=rstd[:])
    nmr = stat_pool.tile([128, 1], F32, tag="nmr")
    nc.vector.tensor_mul(out=nmr[:], in0=mv[:, 0:1], in1=rstd[:])
    nc.vector.tensor_scalar_mul(out=nmr[:], in0=nmr[:], scalar1=-1.0)
    nc.scalar.activation(
        out=xf_t[:], in_=y_t[:], func=ACTF.Identity, bias=nmr[:], scale=rstd[:]
    )


def _build_nc():
    nc = bacc.Bacc(trn_type="TRN2", debug=False, num_devices=W)

    io = {}
    io["xfull"] = nc.dram_tensor("xfull", [S, D], BF, kind="ExternalInput")
    for n in ("wq", "wk", "wv"):
        io[n] = nc.dram_tensor(n, [NP, 128, DB * 128], BF, kind="ExternalInput")
    for n in ("bq", "bk", "bv"):
        io[n] = nc.dram_tensor(n, [NP, 128, 1], F32, kind="ExternalInput")
    io["cos_k"] = nc.dram_tensor("cos_k", [128, S], BF, kind="ExternalInput")
    io["sin_k"] = nc.dram_tensor("sin_k", [128, S], BF, kind="ExternalInput")
    io["cos_q"] = nc.dram_tensor("cos_q", [128, T], BF, kind="ExternalInput")
    io["sin_q"] = nc.dram_tensor("sin_q", [128, T], BF, kind="ExternalInput")
    io["masks"] = nc.dram_tensor("masks", [4, 128, 1024], BF, kind="ExternalInput")
    io["wo"] = nc.dram_tensor("wo", [128, DB * D], BF, kind="ExternalInput")
    io["w1"] = nc.dram_tensor("w1", [FB, 128, DB * 128], BF, kind="ExternalInput")
    io["b1"] = nc.dram_tensor("b1", [128, FB], F32, kind="ExternalInput")
    io["w2"] = nc.dram_tensor("w2", [FB, 128, D], BF, kind="ExternalInput")
    io["b2"] = nc.dram_tensor("b2", [1, D], F32, kind="ExternalInput")
    io["xres"] = nc.dram_tensor("xres", [TPC, D], F32, kind="ExternalInput")
    io["out"] = nc.dram_tensor("out", [TPC, D], F32, kind="ExternalOutput")

    if DEBUG:
        io["dbg_q"] = nc.dram_tensor("dbg_q", [NP, 128, T], BF, kind="ExternalOutput")
        io["dbg_k"] = nc.dram_tensor("dbg_k", [NP, 128, S], BF, kind="ExternalOutput")
        io["dbg_v"] = nc.dram_tensor("dbg_v", [NP, 128, S], BF, kind="ExternalOutput")
        io["dbg_ai"] = nc.dram_tensor(
            "dbg_ai", [2, 128, T], BF, kind="ExternalOutput"
        )
        io["dbg_ao"] = nc.dram_tensor(
            "dbg_ao", [128, DB, TPC], BF, kind="ExternalOutput"
        )
        io["dbg_y"] = nc.dram_tensor("dbg_y", [TPC, D], F32, kind="ExternalOutput")

    io["xa_dram"] = nc.dram_tensor("xa_scratch", [S, D], BF)
    io["xf_dram"] = nc.dram_tensor("xf_scratch", [TPC, D], BF)
    for p_ in range(NP):
        io[f"ag_in{p_}"] = nc.dram_tensor(f"ag_in{p_}", [128, T], BF)
        io[f"ag_out{p_}"] = nc.dram_tensor(
            f"ag_out{p_}", [W * 128, T], BF, addr_space="Shared"
        )

    with tile.TileContext(nc) as tc:
        _emit(nc, tc, io)
    nc.compile()
    return nc


def _emit(nc, tc, io):
    xfull = io["xfull"].ap()
    xf_dram = io["xf_dram"].ap()
    out = io["out"].ap()
    xres = io["xres"].ap()

    # ---- persistent pools ----
    late_cm = tc.tile_pool(name="late", bufs=1)
    stat_cm = tc.tile_pool(name="stats", bufs=8)
    psum_cm = tc.tile_pool(name="psum", bufs=4, space="PSUM")
    psumw_cm = tc.tile_pool(name="psumw", bufs=2, space="PSUM")
    late = late_cm.__enter__()
    stat_pool = stat_cm.__enter__()
    psum = psum_cm.__enter__()
    psumw = psumw_cm.__enter__()

    eps_t = late.tile([128, 1], F32, tag="eps")
    nc.vector.memset(eps_t, EPS)
    b1_t = late.tile([128, FB], F32, tag="b1")
    nc.sync.dma_start(out=b1_t[:], in_=io["b1"].ap())
    b2row = late.tile([1, D], F32, tag="b2row")
    nc.sync.dma_start(out=b2row[:], in_=io["b2"].ap())
    b2_t = late.tile([128, D], F32, tag="b2b")
    nc.gpsimd.partition_broadcast(out_ap=b2_t[:], in_ap=b2row[:])
    wo_t = late.tile([128, DB, D], BF, tag="wo")

    # ---- early constants ----
    early_cm = tc.tile_pool(name="early", bufs=1)
    early = early_cm.__enter__()
    b_tiles = {}
    for n in ("bq", "bk", "bv"):
        b_tiles[n] = early.tile([128, NP, 1], F32, tag=n, name=n + "_t")
        nc.sync.dma_start(
            out=b_tiles[n][:],
            in_=io[n].ap().rearrange("a p c -> p a c"),
        )
    cosk_t = early.tile([128, S], BF, tag="cosk")
    sink_t = early.tile([128, S], BF, tag="sink")
    cosq_t = early.tile([128, T], BF, tag="cosq")
    sinq_t = early.tile([128, T], BF, tag="sinq")
    nc.sync.dma_start(out=cosk_t[:], in_=io["cos_k"].ap())
    nc.sync.dma_start(out=sink_t[:], in_=io["sin_k"].ap())
    nc.sync.dma_start(out=cosq_t[:], in_=io["cos_q"].ap())
    nc.sync.dma_start(out=sinq_t[:], in_=io["sin_q"].ap())
    mask_t = early.tile([128, 4, 1024], BF, tag="masks")
    nc.sync.dma_start(out=mask_t[:], in_=io["masks"].ap().rearrange("j p q -> p j q"))
    w_tiles = {}
    for n in ("wq", "wk", "wv"):
        w_tiles[n] = early.tile([128, NP, DB, 128], BF, tag=n, name=n + "_t")
        nc.sync.dma_start(
            out=w_tiles[n][:],
            in_=io[n].ap().rearrange("a p (d c) -> p a d c", c=128),
        )

    # ---------- phase A: LN1 (this core's batch); xa -> DRAM, then bulk ------
    # transposes per (512-row group x 128-col block), aligned with QKV chunks.
    xa_dram = io["xa_dram"].ap()
    xaT_cm = tc.tile_pool(name="xaT", bufs=1, side="right")
    xaT_pool = xaT_cm.__enter__()
    xaT = xaT_pool.tile([128, DB, S], BF, tag="xaT")
    ln_cm = tc.tile_pool(name="ln", bufs=3)
    ln_pool = ln_cm.__enter__()
    row_groups = []
    r0 = 0
    while r0 < S:
        r1 = min(r0 + 512, S)
        row_groups.append((r0, r1))
        r0 = r1

    def emit_transpose_group(r0, r1):
        for db in range(DB):
            eng = nc.sync if db % 2 == 0 else nc.scalar
            eng.dma_start(
                out=xaT[:, db, r0:r1],
                in_=xa_dram[r0:r1, db * 128 : (db + 1) * 128],
                transpose=True,
            )

    done_rows = 0
    next_grp = 0
    for ch2 in range(S // 256):
        x_t = ln_pool.tile([128, 2, D], BF, tag="xin")
        nc.sync.dma_start(
            out=x_t[:],
            in_=xfull.rearrange("(c p) d -> p c d", p=128)[
                :, ch2 * 2 : ch2 * 2 + 2, :
            ],
        )
        xa_t = ln_pool.tile([128, 2, D], BF, tag="xaout")
        for i in range(2):
            _layernorm_tiles(nc, stat_pool, eps_t, x_t[:, i, :], xa_t[:, i, :])
        nc.scalar.dma_start(
            out=xa_dram.rearrange("(c p) d -> p c d", p=128)[
                :, ch2 * 2 : ch2 * 2 + 2, :
            ],
            in_=xa_t[:],
        )
        done_rows += 256
        while next_grp < len(row_groups) and row_groups[next_grp][1] <= done_rows:
            emit_transpose_group(*row_groups[next_grp])
            next_grp += 1
    while next_grp < len(row_groups):
        emit_transpose_group(*row_groups[next_grp])
        next_grp += 1
    ln_cm.__exit__(None, None, None)

    # ---------- phase C: QKV projections (d-major outputs, per head-pair) ----
    qkv_cm = tc.tile_pool(name="qkvT", bufs=1)
    qkv_pool = qkv_cm.__enter__()
    qT = [qkv_pool.tile([128, T], BF, tag=f"qT{a}", name=f"qT{a}") for a in range(NP)]
    kT = [qkv_pool.tile([128, S], BF, tag=f"kT{a}", name=f"kT{a}") for a in range(NP)]
    vT = [qkv_pool.tile([128, S], BF, tag=f"vT{a}", name=f"vT{a}") for a in range(NP)]

    def proj(w_tile, bias_t, dsts, ranges):
        for grp in range(0, len(ranges), 2):
            chunk = ranges[grp : grp + 2]
            ps = [
                [
                    psum.tile(
                        [128, c[0][1] - c[0][0]], F32, tag="ps", name="ps_proj"
                    )
                    for c in chunk
                ]
                for _a in range(NP)
            ]
            for db in range(DB):
                for a in range(NP):
                    for i, (src, _d) in enumerate(chunk):
                        nc.tensor.matmul(
                            ps[a][i][:],
                            w_tile[:, a, db, :],
                            xaT[:, db, src[0] : src[1]],
                            start=(db == 0),
                            stop=(db == DB - 1),
                        )
            for a in range(NP):
                for i, (_s, dstc) in enumerate(chunk):
                    nc.vector.tensor_scalar_add(
                        out=dsts[a][:, dstc[0] : dstc[1]],
                        in0=ps[a][i][:],
                        scalar1=bias_t[:, a, :],
                    )

    k_chunks = []
    c0 = 0
    while c0 < S:
        c1 = min(c0 + 512, S)
        k_chunks.append(((c0, c1), (c0, c1)))
        c0 = c1
    q_chunks = [
        ((M + i * 512, M + (i + 1) * 512), (i * 512, (i + 1) * 512))
        for i in range(T // 512)
    ]
    # ---------- RoPE via stream_shuffle (rotate-half), 512-col chunks ----------
    rope_cm = tc.tile_pool(name="rope", bufs=4, side="right")
    rope_pool = rope_cm.__enter__()
    SWAP16 = list(range(16, 32)) + list(range(16))

    def rope(dsts, cos_t, sin_t, n):
        for a in range(NP):
            for c0 in range(0, n, 512):
                c1 = min(c0 + 512, n)
                cw = c1 - c0
                rot = rope_pool.tile([128, 512], BF, tag="rot", name="rot")
                t1 = rope_pool.tile([128, 512], BF, tag="t1", name="t1")
                nc.vector.stream_shuffle(
                    out=rot[:, :cw], in_=dsts[a][:, c0:c1], mask=SWAP16
                )
                nc.vector.tensor_mul(
                    out=t1[:, :cw], in0=dsts[a][:, c0:c1], in1=cos_t[:, c0:c1]
                )
                nc.vector.tensor_mul(
                    out=rot[:, :cw], in0=rot[:, :cw], in1=sin_t[:, c0:c1]
                )
                nc.vector.tensor_add(
                    out=dsts[a][:, c0:c1], in0=t1[:, :cw], in1=rot[:, :cw]
                )

    proj(w_tiles["wk"], b_tiles["bk"], kT, k_chunks)
    rope(kT, cosk_t, sink_t, S)
    proj(w_tiles["wq"], b_tiles["bq"], qT, q_chunks)
    rope(qT, cosq_t, sinq_t, T)
    proj(w_tiles["wv"], b_tiles["bv"], vT, k_chunks)
    rope_cm.__exit__(None, None, None)
    xaT_cm.__exit__(None, None, None)


    if DEBUG:
        for a in range(NP):
            nc.sync.dma_start(out=io["dbg_q"].ap()[a], in_=qT[a][:])
            nc.sync.dma_start(out=io["dbg_k"].ap()[a], in_=kT[a][:])
            nc.sync.dma_start(out=io["dbg_v"].ap()[a], in_=vT[a][:])

    # ---------- phase D: V -> token-major via PE transpose ----------
    vtok_cm = tc.tile_pool(name="vtok", bufs=1)
    vtok_pool = vtok_cm.__enter__()
    ident = vtok_pool.tile([128, 128], BF, tag="ident")
    make_identity(nc, ident[:])
    vtok = [
        vtok_pool.tile([128, NKB, 2, 80], BF, tag=f"vtok{a}", name=f"vtok{a}")
        for a in range(NP)
    ]
    for a in range(NP):
        nc.vector.memset(vtok[a][:, :, :, 64:65], 1.0)
        for kb in range(NKB):
            c0 = kb * 128
            for h in range(2):
                pt = psum.tile([128, 64], BF, tag="ps", name="ptr")
                nc.tensor.transpose(
                    pt[:],
                    vT[a][h * DH : (h + 1) * DH, c0 : c0 + 128],
                    ident[h * DH : (h + 1) * DH, h * DH : (h + 1) * DH],
                )
                nc.vector.tensor_copy(out=vtok[a][:, kb, h, 0:64], in_=pt[:])

    # ---------- phase E: attention ----------
    attn_cm = tc.tile_pool(name="attnT", bufs=4)
    attn_pool = attn_cm.__enter__()
    nrm_cm = tc.tile_pool(name="nrm", bufs=4)
    nrm_pool = nrm_cm.__enter__()
    ag_in = [io[f"ag_in{p_}"] for p_ in range(NP)]
    ag_out = [io[f"ag_out{p_}"] for p_ in range(NP)]
    for a in range(NP):
        for qb in range(NQB):
            nk = M // 128 + 4 * (qb + 1)
            qc0 = qb * 512
            po = [psum.tile([65, 512], F32, tag="ps", name="po") for _ in range(2)]
            for kb in range(nk):
                kc0 = kb * 128
                pss = psumw.tile([128, 1024], F32, tag="psw", name="pss")
                for h in range(2):
                    h0 = h * DH
                    nc.tensor.matmul(
                        pss[:, h * 512 : (h + 1) * 512],
                        kT[a][h0 : h0 + DH, kc0 : kc0 + 128],
                        qT[a][h0 : h0 + DH, qc0 : qc0 + 512],
                        start=True,
                        stop=True,
                    )
                aa = attn_pool.tile([128, 1024], BF, tag="at")
                nc.scalar.activation(out=aa[:], in_=pss[:], func=ACTF.Exp)
                jm = kb - (nk - 4)
                if jm >= 0:
                    nc.vector.tensor_mul(out=aa[:], in0=aa[:], in1=mask_t[:, jm, :])
                for h in range(2):
                    nc.tensor.matmul(
                        po[h][:],
                        vtok[a][:, kb, h, 0:65],
                        aa[:, h * 512 : (h + 1) * 512],
                        start=(kb == 0),
                        stop=(kb == nk - 1),
                    )
            for h in range(2):
                rec64 = nrm_pool.tile([65, 512], F32, tag="rec64")
                nc.vector.reciprocal(out=rec64[64:65, :], in_=po[h][64:65, :])
                rec0 = nrm_pool.tile([1, 512], F32, tag="rec0")
                nc.sync.dma_start(out=rec0[:], in_=rec64[64:65, :])
                recb = nrm_pool.tile([64, 512], F32, tag="recb")
                nc.gpsimd.partition_broadcast(out_ap=recb[:], in_ap=rec0[:])
                onorm = nrm_pool.tile([64, 512], BF, tag="onorm")
                nc.vector.tensor_mul(
                    out=onorm[:, :], in0=po[h][0:64, :], in1=recb[:, :]
                )
                nc.sync.dma_start(
                    out=ag_in[a].ap()[h * DH : (h + 1) * DH, qc0 : qc0 + 512],
                    in_=onorm[:, :],
                )
        # publish this pair to all cores (overlaps with the next pair)
        nc.gpsimd.collective_compute(
            "AllGather",
            mybir.AluOpType.bypass,
            replica_groups=[list(range(W))],
            ins=[ag_in[a].ap()],
            outs=[ag_out[a].ap()],
        )
    nrm_cm.__exit__(None, None, None)
    attn_cm.__exit__(None, None, None)
    vtok_cm.__exit__(None, None, None)
    qkv_cm.__exit__(None, None, None)
    early_cm.__exit__(None, None, None)

    if DEBUG:
        nc.sync.dma_start(out=io["dbg_ai"].ap()[0], in_=ag_in[0].ap())
        nc.sync.dma_start(out=io["dbg_ai"].ap()[1], in_=ag_in[1].ap())

    ofT_cm = tc.tile_pool(name="ofT", bufs=1)
    ofT_pool = ofT_cm.__enter__()
    ofT = ofT_pool.tile([128, DB, TPC], BF, tag="ofT")
    pid = nc.gpsimd.partition_id()
    grp4 = pid & 4  # 0 for cores 0-3, 4 for cores 4-7
    qoff = (pid & 3) * 512
    for a in range(NP):
        agv = ag_out[a].ap().rearrange("(s p) q -> p s q", p=128)
        nc.gpsimd.dma_start(
            out=ofT[:, a : DB : 2, :],
            in_=agv[:, bass.ds(grp4, 4), bass.ds(qoff, 512)],
        )
    if DEBUG:
        nc.sync.dma_start(out=io["dbg_ao"].ap(), in_=ofT[:])

    nc.sync.dma_start(
        out=wo_t[:], in_=io["wo"].ap().rearrange("p (a c) -> p a c", c=D)
    )

    # ---------- phase G: out-proj + residual + LN2 ----------
    y_cm = tc.tile_pool(name="y", bufs=1)
    y_pool = y_cm.__enter__()
    xr_cm = tc.tile_pool(name="xr", bufs=2)
    xr_pool = xr_cm.__enter__()
    ln2_cm = tc.tile_pool(name="ln2", bufs=4)
    ln2_pool = ln2_cm.__enter__()
    y_tiles = []
    for tt in range(TPC // 128):
        xr_t = xr_pool.tile([128, D], F32, tag="xr")
        nc.sync.dma_start(out=xr_t[:], in_=xres[tt * 128 : (tt + 1) * 128, :])
        pz = [psum.tile([128, 512], F32, tag="ps", name="pz") for _ in range(2)]
        db_order = [0, 2, 4, 6, 1, 3, 5, 7]
        for di, db in enumerate(db_order):
            for half in range(2):
                nc.tensor.matmul(
                    pz[half][:],
                    ofT[:, db, tt * 128 : (tt + 1) * 128],
                    wo_t[:, db, half * 512 : (half + 1) * 512],
                    start=(di == 0),
                    stop=(di == DB - 1),
                )
        y_t = y_pool.tile([128, D], F32, tag=f"y{tt}", name=f"y{tt}")
        for half in range(2):
            hs = slice(half * 512, (half + 1) * 512)
            nc.vector.tensor_add(out=y_t[:, hs], in0=pz[half][:], in1=xr_t[:, hs])
        y_tiles.append(y_t)
        if DEBUG:
            nc.sync.dma_start(
                out=io["dbg_y"].ap()[tt * 128 : (tt + 1) * 128, :], in_=y_t[:]
            )
        xf_t = ln2_pool.tile([128, D], BF, tag="xf")
        _layernorm_tiles(nc, stat_pool, eps_t, y_t, xf_t)
        nc.sync.dma_start(out=xf_dram[tt * 128 : (tt + 1) * 128, :], in_=xf_t[:])
    ln2_cm.__exit__(None, None, None)
    xr_cm.__exit__(None, None, None)

    # ---------- phase H: transpose xf ----------
    xfT_cm = tc.tile_pool(name="xfT", bufs=1)
    xfT_pool = xfT_cm.__enter__()
    xfT = xfT_pool.tile([128, DB, TPC], BF, tag="xfT")
    for db in range(DB):
        nc.sync.dma_start(
            out=xfT[:, db, :],
            in_=xf_dram[:, db * 128 : (db + 1) * 128],
            transpose=True,
        )

    # ---------- phase I: FFN1 (h = relu(xf@W1+b1)^2, DF-major) ----------
    h2_cm = tc.tile_pool(name="h2T", bufs=1)
    h2_pool = h2_cm.__enter__()
    w2_cm = tc.tile_pool(name="w2full", bufs=1)
    w2_pool = w2_cm.__enter__()
    w1_cm = tc.tile_pool(name="w1c", bufs=3)
    w1_pool = w1_cm.__enter__()
    hr_cm = tc.tile_pool(name="hr", bufs=4)
    hr_pool = hr_cm.__enter__()
    w2full = w2_pool.tile([128, FB, D], BF, tag="w2full")
    nc.sync.dma_start(out=w2full[:], in_=io["w2"].ap().rearrange("f p c -> p f c"))
    h2T = h2_pool.tile([128, FB, TPC], BF, tag="h2T")
    w1_ap = io["w1"].ap()
    for fbg in range(FB // 4):
        w1c = w1_pool.tile([128, 4, DB, 128], BF, tag="w1c")
        nc.sync.dma_start(
            out=w1c[:],
            in_=w1_ap.rearrange("(g f) p (a c) -> p g f a c", f=4, c=128)[
                :, fbg, :, :, :
            ],
        )
        for fi in range(4):
            fb = fbg * 4 + fi
            ph = psum.tile([128, TPC], F32, tag="ps", name="ph")
            for db in range(DB):
                nc.tensor.matmul(
                    ph[:],
                    w1c[:, fi, db, :],
                    xfT[:, db, :],
                    start=(db == 0),
                    stop=(db == DB - 1),
                )
            hr = hr_pool.tile([128, TPC], BF, tag="hr")
            nc.vector.tensor_scalar(
                out=hr[:],
                in0=ph[:],
                scalar1=b1_t[:, fb : fb + 1],
                scalar2=0.0,
                op0=mybir.AluOpType.add,
                op1=mybir.AluOpType.max,
            )
            nc.vector.tensor_mul(out=h2T[:, fb, :], in0=hr[:], in1=hr[:])
    hr_cm.__exit__(None, None, None)
    w1_cm.__exit__(None, None, None)

    # ---------- phase J: FFN2 + residual + store ----------
    out_cm = tc.tile_pool(name="outp", bufs=2)
    out_pool = out_cm.__enter__()
    for tt in range(TPC // 128):
        pz = [psum.tile([128, 512], F32, tag="ps", name="pz2") for _ in range(2)]
        for fb in range(FB):
            for half in range(2):
                nc.tensor.matmul(
                    pz[half][:],
                    h2T[:, fb, tt * 128 : (tt + 1) * 128],
                    w2full[:, fb, half * 512 : (half + 1) * 512],
                    start=(fb == 0),
                    stop=(fb == FB - 1),
                )
        o_t = out_pool.tile([128, D], F32, tag="ot")
        for half in range(2):
            hs = slice(half * 512, (half + 1) * 512)
            nc.vector.tensor_add(
                out=o_t[:, hs], in0=pz[half][:], in1=y_tiles[tt][:, hs]
            )
            nc.vector.tensor_add(out=o_t[:, hs], in0=o_t[:, hs], in1=b2_t[:, hs])
        nc.sync.dma_start(out=out[tt * 128 : (tt + 1) * 128, :], in_=o_t[:])
    out_cm.__exit__(None, None, None)
    w1_cm2 = None  # noqa
    w2_cm.__exit__(None, None, None)
    h2_cm.__exit__(None, None, None)
    xfT_cm.__exit__(None, None, None)
    y_cm.__exit__(None, None, None)
    ofT_cm.__exit__(None, None, None)
    stat_cm.__exit__(None, None, None)
    late_cm.__exit__(None, None, None)
    psumw_cm.__exit__(None, None, None)
    psum_cm.__exit__(None, None, None)


def _prep_inputs(x, memory, Wq, bq, Wk, bk, Wv, bv, Wo, bo, W1, b1, W2, b2,
                 ln1_g, ln1_b, ln2_g, ln2_b):
    f32 = np.float32
    x = np.asarray(x, f32)
    memory = np.asarray(memory, f32)
    x_full_b = np.concatenate([memory, x], axis=1)  # (B, S, D)

    g1 = np.asarray(ln1_g, f32)
    b1n = np.asarray(ln1_b, f32)
    g2 = np.asarray(ln2_g, f32)
    b2n = np.asarray(ln2_b, f32)

    scale_q = np.float32(1.0 / np.sqrt(np.float32(DH)))
    Wq_e = (g1[:, None] * np.asarray(Wq, f32)) * scale_q
    bq_e = (b1n @ np.asarray(Wq, f32) + np.asarray(bq, f32)) * scale_q
    Wk_e = g1[:, None] * np.asarray(Wk, f32)
    bk_e = b1n @ np.asarray(Wk, f32) + np.asarray(bk, f32)
    Wv_e = g1[:, None] * np.asarray(Wv, f32)
    bv_e = b1n @ np.asarray(Wv, f32) + np.asarray(bv, f32)
    W1_e = g2[:, None] * np.asarray(W1, f32)
    b1_e = b2n @ np.asarray(W1, f32) + np.asarray(b1, f32)

    inv_freq = 1.0 / (10000.0 ** (np.arange(0, R, 2, dtype=f32) / np.float32(R)))
    t = np.arange(S, dtype=f32)
    freqs = t[:, None] * inv_freq[None, :]
    emb = np.concatenate([freqs, freqs], axis=-1)  # (S, R)
    cos = np.cos(emb).T.astype(f32)  # (R, S)
    sin = np.sin(emb).T.astype(f32)
    ssin = np.concatenate([-sin[:16], sin[16:]], axis=0)

    def wide(tab, fill):
        o = np.full((128, tab.shape[1]), fill, np.float32)
        o[0:R] = tab
        o[64 : 64 + R] = tab
        return o

    cos_k = wide(cos, 1.0).astype(BF16)
    sin_k = wide(ssin, 0.0).astype(BF16)
    cos_q = wide(cos[:, M:], 1.0).astype(BF16)
    sin_q = wide(ssin[:, M:], 0.0).astype(BF16)

    kk = np.arange(128)[:, None]
    qq = np.arange(512)[None, :]
    mask = np.stack([(qq >= 128 * j + kk) for j in range(4)]).astype(BF16)
    mask = np.concatenate([mask, mask], axis=2)  # (4, 128, 1024) both heads

    wo_host = (
        np.ascontiguousarray(
            np.asarray(Wo, f32).reshape(DB, 128, D).transpose(1, 0, 2)
        )
        .reshape(128, DB * D)
        .astype(BF16)
    )
    w1_host = (
        np.ascontiguousarray(W1_e.reshape(DB, 128, FB, 128).transpose(2, 1, 0, 3))
        .reshape(FB, 128, DB * 128)
        .astype(BF16)
    )
    w2_host = np.asarray(W2, f32).reshape(FB, 128, D).astype(BF16)
    b1_host = np.ascontiguousarray(b1_e.reshape(FB, 128).T).astype(f32)
    b2_host = np.ascontiguousarray(np.asarray(b2, f32).reshape(1, D))

    bo_arr = np.asarray(bo, f32)
    x_dec = x.reshape(BT, D)

    in_maps = []
    for c in range(W):
        g = c // 4  # batch handled by this core
        p = c % 4  # group position -> global heads 4p..4p+4
        hcols = slice(p * 256, (p + 1) * 256)

        def wslice(We):
            # (1024, 256) -> (NP, 128, DB*128), head-pair major
            wc = We[:, hcols].reshape(DB, 128, NP, 128)
            return (
                np.ascontiguousarray(wc.transpose(2, 1, 0, 3))
                .reshape(NP, 128, DB * 128)
                .astype(BF16)
            )

        def bslice(be):
            return np.ascontiguousarray(be[hcols].reshape(NP, 128, 1), dtype=f32)

        xres_c = x_dec[c * TPC : (c + 1) * TPC] + bo_arr[None, :]
        in_maps.append(
            {
                "xfull": x_full_b[g].astype(BF16),
                "wq": wslice(Wq_e),
                "wk": wslice(Wk_e),
                "wv": wslice(Wv_e),
                "bq": bslice(bq_e),
                "bk": bslice(bk_e),
                "bv": bslice(bv_e),
                "cos_k": cos_k,
                "sin_k": sin_k,
                "cos_q": cos_q,
                "sin_q": sin_q,
                "masks": mask,
                "wo": wo_host,
                "w1": w1_host,
                "b1": b1_host,
                "w2": w2_host,
                "b2": b2_host,
                "xres": np.ascontiguousarray(xres_c, dtype=f32),
            }
        )
    return in_maps


def kernel(**inputs):
    global _CACHED_NC
    if _CACHED_NC is None:
        _CACHED_NC = _build_nc()
    nc = _CACHED_NC
    in_maps = _prep_inputs(**inputs)
    res = bass_utils.run_bass_kernel_spmd(nc, in_maps, core_ids=list(range(W)))
    KERNEL_STATS["exec_time_ns"] = res.exec_time_ns
    outs = np.stack([res.results[c]["out"] for c in range(W)])  # (W, TPC, D)
    return outs.reshape(B, T, D).astype(np.float32)
